# revision 24
# baseline (speedup 1.0000x reference)
"""Multi-head attention (B=2, S=2048, E=1024, H=16, D=64) on 8 TRN2 cores.

Sharding: tensor-parallel over heads. Core c owns heads {2c, 2c+1}:
  - Q/K/V projections column-sharded (128 cols each per core)
  - attention for the core's 2 heads (both batches)
  - out-projection row-sharded (128 rows of Wo) -> partial [4096,1024]
  - host sums the 8 partials and adds bo.

On-chip layout (everything "transposed"):
  - host passes xT [1024, 4096] (E-major, fp16) so the contraction dim
    lands on SBUF partitions with no on-device transpose of x
  - projections produce Q^T, K^T [128, 4096] with head0 on partitions
    0:64 and head1 on 64:128, and V^T, which is PE-transposed to
    token-major V tiles (64 cols + ones col per head)
  - scores are computed transposed: scores^T[kk, q] so softmax's key
    reduction can ride the attn@V matmul (ones-column in V) and the
    key-padding mask folds into the exp() per-partition bias

Perf structure (v4):
  - the steady state is an exp()-per-key-tile pipeline on ACT; one exp
    covers both heads [128,1024] (the mask bias is per-key, shared)
  - the PE instruction stream executes in order, so attn@V is emitted
    ONE TILE BEHIND its exp: the stream is [scores(t) | attnV(t-1) |
    filler | scores(t+1) ...] and never head-of-line blocks on ACT
  - scores for the two heads run concurrently as PE row-tiles
    (tile_position (0,0)/(64,0), K=64 each) writing adjacent PSUM banks
  - ALL projection work and the out-projection are chopped into ~2-MM
    pieces interleaved into the per-tile PE slack, scheduled by a
    computed just-in-time demand table (emission order = dataflow)
  - out-projection uses the stacked Y^T [128, M] (both heads): one
    K=128 matmul per 512 output columns against stacked Wo rows
  - PE warmup matmuls run during the initial DMA window so the HAM
    clock is at full rate when real matmuls start; a few out-proj
    tiles are reserved to hide the final normalization's DMA latency
  - PSUM: 4 banks scores (double-buffered) + 2 banks attn@V
    accumulators + 2 filler banks
"""

import os
import numpy as np

B, S, E, H, D = 2, 2048, 1024, 16, 64
M = B * S            # 4096 tokens
P = 128              # partitions
NCORES = 8
KC = E // P          # 8 contraction chunks for projections
MCH = 512            # token chunk for projections
QCH = 512            # query chunk for attention
NKT = S // P         # 16 key tiles per batch
NMC = M // MCH       # 8 projection token chunks
NEG = -1.0e30

LAST_RESULTS = None  # BassKernelResults of the most recent run (for test harness)
_PROGRAM = None


def _build_program():
    import concourse.bass as bass
    import concourse.tile as tile
    from concourse import bacc, mybir
    from concourse.masks import make_identity

    f32 = mybir.dt.float32
    f16 = mybir.dt.float16

    nc = bacc.Bacc(
        "TRN2",
        target_bir_lowering=False,
        debug=False,
        enable_asserts=False,
        num_devices=NCORES,
    )

    xT_d = nc.dram_tensor("xT", (E, M), f16, kind="ExternalInput").ap()
    wq_d = nc.dram_tensor("wq", (P, KC, P), f16, kind="ExternalInput").ap()
    wk_d = nc.dram_tensor("wk", (P, KC, P), f16, kind="ExternalInput").ap()
    wv_d = nc.dram_tensor("wv", (P, KC, P), f16, kind="ExternalInput").ap()
    wo_d = nc.dram_tensor("wo", (P, E), f16, kind="ExternalInput").ap()
    bq_d = nc.dram_tensor("bq", (P, 1), f32, kind="ExternalInput").ap()
    bk_d = nc.dram_tensor("bk", (P, 1), f32, kind="ExternalInput").ap()
    bv_d = nc.dram_tensor("bv", (P, 1), f32, kind="ExternalInput").ap()
    maskT_d = nc.dram_tensor("maskT", (P, B * 16), f32, kind="ExternalInput").ap()
    out_d = nc.dram_tensor("out", (M, E), f16, kind="ExternalOutput").ap()

    with tile.TileContext(nc) as tc:
        with (
            tc.tile_pool(name="consts", bufs=1) as consts,
            tc.tile_pool(name="big", bufs=1) as big,
            tc.tile_pool(name="xt_pool", bufs=26) as xt_pool,
            tc.tile_pool(name="vt_pool", bufs=2) as vt_pool,
            tc.tile_pool(name="pt_pool", bufs=4) as pt_pool,
            tc.tile_pool(name="r_pool", bufs=2) as r_pool,
            tc.tile_pool(name="out_pool", bufs=6) as out_pool,
            tc.tile_pool(name="psum_sc", bufs=2, space="PSUM") as psum_sc,
            tc.tile_pool(name="psum_av", bufs=2, space="PSUM") as psum_av,
            tc.tile_pool(name="psum_f", bufs=2, space="PSUM") as psum_f,
        ):
            # ---- constants ----
            wq_sb = consts.tile([P, KC, P], f16)
            wk_sb = consts.tile([P, KC, P], f16)
            wv_sb = consts.tile([P, KC, P], f16)
            wo_sb = consts.tile([P, E], f16)
            bq_sb = consts.tile([P, 1], f32)
            bk_sb = consts.tile([P, 1], f32)
            bv_sb = consts.tile([P, 1], f32)
            mask_sb = consts.tile([P, B * 16], f32)
            ident = consts.tile([P, P], f16)
            ones_h = consts.tile([P, M // P], f16)
            ones_bc = consts.tile([P, D], f16)   # broadcast stationary
            wu_sb = consts.tile([P, QCH], f16)
            tiny = consts.tile([P, 1], f32)
            tiny2 = consts.tile([P, 1], f32)

            # first weights on the fast queue (gates the first matmul),
            # remaining constants on the SWDGE queue behind it
            nc.sync.dma_start(wq_sb, wq_d)
            nc.gpsimd.dma_start(wk_sb, wk_d)
            nc.gpsimd.dma_start(mask_sb, maskT_d)
            nc.gpsimd.dma_start(wv_sb, wv_d)
            nc.gpsimd.dma_start(wo_sb, wo_d)
            nc.gpsimd.dma_start(bq_sb, bq_d)
            nc.gpsimd.dma_start(bk_sb, bk_d)
            nc.gpsimd.dma_start(bv_sb, bv_d)
            nc.vector.memset(wu_sb, 0.0)
            nc.vector.memset(tiny, 0.0)
            make_identity(nc, ident)
            nc.vector.memset(ones_h, 1.0)
            nc.vector.memset(ones_bc, 1.0)

            # ---- big persistent activations ----
            QT = big.tile([P, M], f16)       # head0 dims on parts 0:64, head1 on 64:128
            KT = big.tile([P, M], f16)
            # token-major V tiles: [tok, mt, 2*(64 cols + ones col)]
            Vtm = big.tile([P, M // P, 2 * (D + 1)], f16)
            YT = big.tile([P, M], f16)       # stacked attention output^T (both heads)

            ones_col = ones_h[:, 0 : M // P].rearrange("p (a b) -> p a b", b=1)
            nc.vector.tensor_copy(Vtm[:, :, D : D + 1], ones_col)
            nc.vector.tensor_copy(Vtm[:, :, 2 * D + 1 : 2 * D + 2], ones_col)

            Exp = mybir.ActivationFunctionType.Exp

            # PE warmup: junk matmuls on a memset tile so the HAM clock
            # reaches full rate before the first real matmul (cold-start
            # matmuls otherwise run ~2-4x slower); also preload the ACT
            # exp table (one-time ~1.5us) while DMA queues spin up.
            wu_ps = psum_f.tile([P, QCH], f32, tag="f", name="wu")
            for _ in range(12):
                nc.tensor.matmul(wu_ps, wu_sb[:, 0:P], wu_sb, start=True, stop=True)
            nc.scalar.activation(tiny2, tiny, Exp, bias=0.0, scale=1.0)

            # ---- projection pieces: each token chunk mc yields small
            # filler closures (~2 matmuls each) so pops never delay the
            # exp stream by more than ~0.5us ----
            xts_by_mc = {}

            def make_pieces(mc, dual_queue=False):
                """Returns (kx_pieces[5] (prefetch + 4), vt_pieces[8],
                q_pieces[4])."""
                msl = bass.ts(mc, MCH)
                st = {}

                def pf(mc=mc, msl=msl):
                    # DMA-issue only: popped several tiles ahead of the
                    # matmuls so the x chunk is resident when they run
                    xts = []
                    for kc in range(KC):
                        xt = xt_pool.tile([P, MCH], f16, tag="xt", name="xt")
                        eng = nc.scalar if (dual_queue and kc % 2) else nc.sync
                        eng.dma_start(xt, xT_d[bass.ts(kc, P), msl])
                        xts.append(xt)
                    xts_by_mc[mc] = xts

                def kx(i, mc=mc, msl=msl, st=st):
                    if i == 0:
                        st["kp"] = psum_f.tile([P, MCH], f32, tag="f", name="kp")
                    for kc in (2 * i, 2 * i + 1):
                        nc.tensor.matmul(
                            st["kp"], wk_sb[:, kc, :], xts_by_mc[mc][kc],
                            start=(kc == 0), stop=(kc == KC - 1),
                        )
                    if i == 3:
                        nc.vector.tensor_scalar_add(KT[:, msl], st["kp"], bk_sb)

                def vt(i, mc=mc, msl=msl, st=st):
                    if i < 4:
                        if i == 0:
                            st["vp"] = psum_f.tile([P, MCH], f32, tag="f", name="vp")
                        for kc in (2 * i, 2 * i + 1):
                            nc.tensor.matmul(
                                st["vp"], wv_sb[:, kc, :], xts_by_mc[mc][kc],
                                start=(kc == 0), stop=(kc == KC - 1),
                            )
                        if i == 3:
                            v = vt_pool.tile([P, MCH], f16, name="vt", tag="vt")
                            nc.vector.tensor_scalar_add(v, st["vp"], bv_sb)
                            st["vt"] = v
                    else:
                        j = i - 4
                        mt = mc * (MCH // P) + j
                        vtp = psum_f.tile([P, P], f16, tag="f", name="vtp")
                        nc.tensor.transpose(vtp, st["vt"][:, bass.ts(j, P)], ident)
                        nc.vector.tensor_copy(Vtm[:, mt, 0:D], vtp[:, 0:D])
                        nc.vector.tensor_copy(
                            Vtm[:, mt, D + 1 : 2 * D + 1], vtp[:, D : 2 * D]
                        )

                def q(i, mc=mc, msl=msl, st=st):
                    if i == 0:
                        st["qp"] = psum_f.tile([P, MCH], f32, tag="f", name="qp")
                    for kc in (2 * i, 2 * i + 1):
                        nc.tensor.matmul(
                            st["qp"], wq_sb[:, kc, :], xts_by_mc[mc][kc],
                            start=(kc == 0), stop=(kc == KC - 1),
                        )
                    if i == 3:
                        nc.vector.tensor_scalar_add(QT[:, msl], st["qp"], bq_sb)
                        del xts_by_mc[mc]

                KXp = [pf] + [(lambda i=i, f=kx: f(i)) for i in range(4)]
                VTp = [(lambda i=i, f=vt: f(i)) for i in range(8)]
                Qp = [(lambda i=i, f=q: f(i)) for i in range(4)]
                return KXp, VTp, Qp

            pieces = [make_pieces(mc, dual_queue=(mc == 0)) for mc in range(NMC)]
            # batch-0 chunk 0 inline: the first pass starts right after
            for p in pieces[0][0] + pieces[0][1] + pieces[0][2]:
                p()

            # filler order (emission order defines dataflow):
            #   b0 chunks 1-3 K/V just-in-time for pass 0, then b0 Q
            #   chunks, then all b1 chunks, then out-proj tiles (appended
            #   as their normalization completes)
            filler = []
            idx = {}

            def add(name, ps):
                idx[name] = len(filler) + len(ps)   # index AFTER this group
                filler.extend(ps)

            for mc in (1, 2, 3):
                add(f"kx{mc}", pieces[mc][0])
                add(f"vt{mc}", pieces[mc][1])
            for mc in (1, 2, 3):
                add(f"q{mc}", pieces[mc][2])
            for mc in (4, 5, 6, 7):
                add(f"kx{mc}", pieces[mc][0])
                add(f"vt{mc}", pieces[mc][1])
                add(f"q{mc}", pieces[mc][2])
            popped = [0]

            def pop(n=1):
                for _ in range(n):
                    if filler:
                        filler.pop(0)()
                        popped[0] += 1

            def pop_until(k):
                while popped[0] < k and filler:
                    pop()

            # just-in-time demand: before emitting iteration (pi, t),
            # this many filler pieces must have been emitted (emission
            # order defines the data a read sees).
            def demand(pi, t):
                b, qc = passes[pi]
                need = 0
                # Q chunks consumed by this pass's scores
                qmax = qc if b == 0 else 4 + qc
                for k in range(1, qmax + 1):
                    need = max(need, idx[f"q{k}"])
                # progressive K/V within the first pass of each batch:
                # scores(t) reads KT chunk ck; the attn@V flushed at this
                # iteration reads Vtm transpose piece for tile t-1 (use t:
                # one ahead, also covers the cross-pass flush)
                if qc == 0:
                    lo = 1 if b == 0 else 4
                    hi = 3 if b == 0 else 7
                    ck = t // 4 + 4 * b
                    if lo <= ck:
                        need = max(need, idx[f"kx{ck}"])
                        base = idx[f"vt{ck}"] - 8
                        need = max(need, base + 4 + (t % 4) + 1)
                    # prefetch: next chunk's x DMAs issued 4 tiles ahead
                    ck2 = ck + 1
                    if lo <= ck2 <= hi:
                        need = max(need, idx[f"kx{ck2}"] - 4)
                # prefetch chunk 4's x before the b0->b1 transition
                if pi + 1 < len(passes) and passes[pi + 1] == (1, 0) and t >= 12:
                    need = max(need, idx["kx4"] - 4)
                return need

            def emit_norm_h(b, qc, h, av_sb):
                qsl = bass.ds(b * S + qc * QCH, QCH)
                # partition-broadcast the raw sums [1,512] -> [64,512] with
                # a K=1 PE matmul (ones stationary), then reciprocal at base
                # partition 0 (custom-DVE approx ops misbehave at base 64)
                # and the normalize multiply on the otherwise-idle GPSIMD.
                s16 = r_pool.tile([1, QCH], f16, tag="s16", bufs=4, name="s16")
                nc.vector.tensor_copy(s16, av_sb[D : D + 1, :])
                bc = psum_f.tile([D, QCH], f32, tag="f", name="bc")
                nc.tensor.matmul(bc, ones_bc[0:1, :], s16, start=True, stop=True)
                rbs = r_pool.tile([D, QCH], f32, tag="rbs", bufs=4, name="rbs")
                rsc2 = r_pool.tile([D, QCH], f32, tag="rsc2", name="rsc2")
                nc.vector.reciprocal_approx_accurate(rbs, bc, rsc2)
                nc.gpsimd.tensor_mul(
                    YT[bass.ds(D * h, D), qsl], av_sb[0:D, :], rbs
                )

            def emit_norm(b, qc, av_sbs):
                for h in range(2):
                    emit_norm_h(b, qc, h, av_sbs[h])

            def emit_outproj_ec(b, j, ec, spare_psum=False, stage_eng=None):
                m0 = b * S + j * P
                esl = bass.ts(ec, 512)
                if spare_psum:
                    op = psum_sc.tile([P, 512], f32, tag="sc", name="op_s")
                else:
                    op = psum_f.tile([P, 512], f32, tag="f", name="op")
                nc.tensor.matmul(
                    op, YT[:, bass.ds(m0, P)], wo_sb[:, esl], start=True, stop=True
                )
                osb = out_pool.tile([P, 512], f16, name="osb")
                if stage_eng == "scalar":
                    nc.scalar.copy(osb, op)
                else:
                    nc.vector.tensor_copy(osb, op)
                nc.sync.dma_start(out_d[bass.ds(m0, P), esl], osb)

            def outproj_pieces(b, jlo, jhi):
                return [
                    (lambda bb=b, jj=j, ee=ec: emit_outproj_ec(bb, jj, ee))
                    for j in range(jlo, jhi)
                    for ec in range(E // 512)
                ]

            # ---- attention passes, attn@V pipelined one tile behind ----
            # batch-1 is pulled forward so its projection drain and its
            # out-proj tiles spread across more passes
            passes = [(0, 0), (0, 1), (0, 2), (1, 0), (0, 3), (1, 1), (1, 2), (1, 3)]
            pending = []
            done = {0: set(), 1: set()}
            deferred = [None]

            def flush_av():
                if deferred[0] is not None:
                    deferred[0]()
                    deferred[0] = None

            for pi, (b, qc) in enumerate(passes):
                qsl = bass.ds(b * S + qc * QCH, QCH)
                avs = [
                    psum_av.tile([D + 1, QCH], f32, tag="av", name="av")
                    for _ in range(2)
                ]
                for t in range(NKT):
                    pop_until(demand(pi, t))
                    ksl = bass.ds(b * S + t * P, P)
                    bt = b * 16 + t
                    # scores for both heads concurrently: PE row tiles
                    # (0,0) and (64,0) writing adjacent PSUM banks
                    sc = psum_sc.tile([P, 2 * QCH], f32, tag="sc", name="sc")
                    nc.tensor.matmul(
                        sc[:, 0:QCH], KT[0:D, ksl], QT[0:D, qsl],
                        start=True, stop=True,
                    )
                    nc.tensor.matmul(
                        sc[:, QCH : 2 * QCH], KT[D:P, ksl], QT[D:P, qsl],
                        start=True, stop=True,
                    )
                    # one exp() covers both heads: the key-padding bias
                    # is per-key (partition), identical for both heads
                    pt = pt_pool.tile([P, 2 * QCH], f16, tag="pt", name="pt")
                    nc.scalar.activation(
                        pt, sc, Exp, bias=mask_sb[:, bt : bt + 1], scale=1.0
                    )
                    # attn@V of the PREVIOUS tile (its exp is long done, so
                    # the in-order PE stream never waits on ACT here)
                    flush_av()

                    def av_emit(avs=avs, bt=bt, pt=pt, first=(t == 0),
                                last=(t == NKT - 1), pp=(b, qc)):
                        nc.tensor.matmul(
                            avs[0], Vtm[:, bt, 0 : D + 1], pt[:, 0:QCH],
                            start=first, stop=last,
                        )
                        nc.tensor.matmul(
                            avs[1], Vtm[:, bt, D + 1 : 2 * (D + 1)],
                            pt[:, QCH : 2 * QCH],
                            start=first, stop=last,
                        )
                        if last:
                            av_sbs = []
                            for h in range(2):
                                av_sb = r_pool.tile(
                                    [D + 1, QCH], f32, tag="avsb", bufs=4,
                                    name="avsb",
                                )
                                nc.vector.tensor_copy(av_sb, avs[h])
                                av_sbs.append(av_sb)
                            pending.append((pp[0], pp[1], av_sbs))

                    deferred[0] = av_emit
                    # previous pass's normalization (DVE/DMA only)
                    if t == 1 and pending:
                        nb, nqc, nav = pending.pop(0)
                        emit_norm(nb, nqc, nav)
                        done[nb].add(nqc)
                        if nb == 0 and len(done[0]) == 4:
                            filler.extend(outproj_pieces(0, 0, S // P))
                        elif nb == 1 and nqc < 3:
                            # qc2's last two tiles are reserved for the tail
                            hi = 4 * (nqc + 1) if nqc != 2 else 4 * nqc + 2
                            filler.extend(outproj_pieces(1, 4 * nqc, hi))
                    # steady drip of filler into the per-tile PE slack;
                    # passes 1-3 drain harder so the b1 projections are
                    # fully resident before the b0->b1 transition
                    if t not in (0, NKT - 1):
                        pop(2 if 1 <= pi <= 3 else 1)
            # tail: flush the last attn@V, issue the last normalization's
            # DMA/reciprocal chain, hide its latency behind the remaining
            # filler, then the final out-proj tiles it unblocks
            flush_av()
            b_l, qc_l, av_sbs_l = pending.pop(0)   # (1, 3)
            emit_norm_h(b_l, qc_l, 0, av_sbs_l[0])
            emit_norm_h(b_l, qc_l, 1, av_sbs_l[1])
            while filler:
                pop()
            k = 0
            for j in range(10, S // P):
                for ec in range(E // 512):
                    emit_outproj_ec(
                        1, j, ec,
                        spare_psum=(j >= 12 and ec % 2 == 1),
                        stage_eng="scalar" if k % 2 else None,
                    )
                    k += 1

    nc.compile()
    return nc


def kernel(x, mask, Wq, bq, Wk, bk, Wv, bv, Wo, bo):
    global LAST_RESULTS, _PROGRAM
    from concourse.bass_utils import run_bass_kernel_spmd

    if _PROGRAM is None:
        _PROGRAM = _build_program()
    nc = _PROGRAM

    f16 = np.float16
    x = np.asarray(x, dtype=np.float32)
    mask = np.asarray(mask)
    f32c = lambda a: np.ascontiguousarray(np.asarray(a, dtype=np.float32))

    xT = np.ascontiguousarray(x.reshape(M, E).T.astype(f16))     # [E, M]
    maskf = np.where(mask, np.float32(NEG), np.float32(0.0)).astype(np.float32)
    maskT = np.ascontiguousarray(
        maskf.reshape(B, 16, P).transpose(2, 0, 1).reshape(P, B * 16)
    )
    scale = np.float32(1.0 / np.sqrt(D))

    in_maps = []
    for c in range(NCORES):
        csl = slice(P * c, P * (c + 1))
        wq_c = (np.asarray(Wq, dtype=np.float32)[:, csl] * scale).astype(f16)
        wk_c = np.asarray(Wk, dtype=np.float32)[:, csl].astype(f16)
        wv_c = np.asarray(Wv, dtype=np.float32)[:, csl].astype(f16)
        in_maps.append(
            {
                "xT": xT,
                "wq": np.ascontiguousarray(wq_c.reshape(KC, P, P).transpose(1, 0, 2)),
                "wk": np.ascontiguousarray(wk_c.reshape(KC, P, P).transpose(1, 0, 2)),
                "wv": np.ascontiguousarray(wv_c.reshape(KC, P, P).transpose(1, 0, 2)),
                "wo": np.ascontiguousarray(
                    np.asarray(Wo, dtype=np.float32)[csl, :].astype(f16)
                ),
                "bq": f32c(np.asarray(bq)[csl] * scale).reshape(P, 1),
                "bk": f32c(np.asarray(bk)[csl]).reshape(P, 1),
                "bv": f32c(np.asarray(bv)[csl]).reshape(P, 1),
                "maskT": maskT,
            }
        )

    trace = bool(os.environ.get("KERNEL_TRACE"))
    LAST_RESULTS = run_bass_kernel_spmd(
        nc, in_maps, list(range(NCORES)), trace=trace
    )

    acc = np.zeros((M, E), dtype=np.float32)
    for res in LAST_RESULTS.results:
        acc += res["out"].astype(np.float32)
    out = (acc + np.asarray(bo, dtype=np.float32)[None, :]).astype(np.float32)
    return out.reshape(B, S, E)


# revision 25
# speedup vs baseline: 1.0161x; 1.0161x over previous
"""Multi-head attention (B=2, S=2048, E=1024, H=16, D=64) on 8 TRN2 cores.

Sharding: tensor-parallel over heads. Core c owns heads {2c, 2c+1}:
  - Q/K/V projections column-sharded (128 cols each per core)
  - attention for the core's 2 heads (both batches)
  - out-projection row-sharded (128 rows of Wo) -> partial [4096,1024]
  - host sums the 8 partials and adds bo.

On-chip layout (everything "transposed"):
  - host passes xT [1024, 4096] (E-major, fp16) so the contraction dim
    lands on SBUF partitions with no on-device transpose of x
  - projections produce Q^T, K^T [128, 4096] with head0 on partitions
    0:64 and head1 on 64:128, and V^T, which is PE-transposed to
    token-major V tiles (64 cols + ones col per head)
  - scores are computed transposed: scores^T[kk, q] so softmax's key
    reduction can ride the attn@V matmul (ones-column in V) and the
    key-padding mask folds into the exp() per-partition bias

Perf structure (v4):
  - the steady state is an exp()-per-key-tile pipeline on ACT; one exp
    covers both heads [128,1024] (the mask bias is per-key, shared)
  - the PE instruction stream executes in order, so attn@V is emitted
    ONE TILE BEHIND its exp: the stream is [scores(t) | attnV(t-1) |
    filler | scores(t+1) ...] and never head-of-line blocks on ACT
  - scores for the two heads run concurrently as PE row-tiles
    (tile_position (0,0)/(64,0), K=64 each) writing adjacent PSUM banks
  - ALL projection work and the out-projection are chopped into ~2-MM
    pieces interleaved into the per-tile PE slack, scheduled by a
    computed just-in-time demand table (emission order = dataflow)
  - out-projection uses the stacked Y^T [128, M] (both heads): one
    K=128 matmul per 512 output columns against stacked Wo rows
  - PE warmup matmuls run during the initial DMA window so the HAM
    clock is at full rate when real matmuls start; a few out-proj
    tiles are reserved to hide the final normalization's DMA latency
  - PSUM: 4 banks scores (double-buffered) + 2 banks attn@V
    accumulators + 2 filler banks
"""

import os
import numpy as np

B, S, E, H, D = 2, 2048, 1024, 16, 64
M = B * S            # 4096 tokens
P = 128              # partitions
NCORES = 8
KC = E // P          # 8 contraction chunks for projections
MCH = 512            # token chunk for projections
QCH = 512            # query chunk for attention
NKT = S // P         # 16 key tiles per batch
NMC = M // MCH       # 8 projection token chunks
NEG = -1.0e30

LAST_RESULTS = None  # BassKernelResults of the most recent run (for test harness)
_PROGRAM = None


def _build_program():
    import concourse.bass as bass
    import concourse.tile as tile
    from concourse import bacc, mybir
    from concourse.masks import make_identity

    f32 = mybir.dt.float32
    f16 = mybir.dt.float16

    nc = bacc.Bacc(
        "TRN2",
        target_bir_lowering=False,
        debug=False,
        enable_asserts=False,
        num_devices=NCORES,
    )

    xT_d = nc.dram_tensor("xT", (E, M), f16, kind="ExternalInput").ap()
    wq_d = nc.dram_tensor("wq", (P, KC, P), f16, kind="ExternalInput").ap()
    wk_d = nc.dram_tensor("wk", (P, KC, P), f16, kind="ExternalInput").ap()
    wv_d = nc.dram_tensor("wv", (P, KC, P), f16, kind="ExternalInput").ap()
    wo_d = nc.dram_tensor("wo", (P, E), f16, kind="ExternalInput").ap()
    bq_d = nc.dram_tensor("bq", (P, 1), f32, kind="ExternalInput").ap()
    bk_d = nc.dram_tensor("bk", (P, 1), f32, kind="ExternalInput").ap()
    bv_d = nc.dram_tensor("bv", (P, 1), f32, kind="ExternalInput").ap()
    maskT_d = nc.dram_tensor("maskT", (P, B * 16), f32, kind="ExternalInput").ap()
    out_d = nc.dram_tensor("out", (M, E), f16, kind="ExternalOutput").ap()

    with tile.TileContext(nc) as tc:
        with (
            tc.tile_pool(name="consts", bufs=1) as consts,
            tc.tile_pool(name="big", bufs=1) as big,
            tc.tile_pool(name="xt_pool", bufs=26) as xt_pool,
            tc.tile_pool(name="vt_pool", bufs=2) as vt_pool,
            tc.tile_pool(name="pt_pool", bufs=4) as pt_pool,
            tc.tile_pool(name="r_pool", bufs=2) as r_pool,
            tc.tile_pool(name="out_pool", bufs=6) as out_pool,
            tc.tile_pool(name="psum_sc", bufs=2, space="PSUM") as psum_sc,
            tc.tile_pool(name="psum_av", bufs=2, space="PSUM") as psum_av,
            tc.tile_pool(name="psum_f", bufs=2, space="PSUM") as psum_f,
        ):
            # ---- constants ----
            wq_sb = consts.tile([P, KC, P], f16)
            wk_sb = consts.tile([P, KC, P], f16)
            wv_sb = consts.tile([P, KC, P], f16)
            wo_sb = consts.tile([P, E], f16)
            bq_sb = consts.tile([P, 1], f32)
            bk_sb = consts.tile([P, 1], f32)
            bv_sb = consts.tile([P, 1], f32)
            mask_sb = consts.tile([P, B * 16], f32)
            ident = consts.tile([P, P], f16)
            ones_h = consts.tile([P, M // P], f16)
            ones_bc = consts.tile([P, D], f16)   # broadcast stationary
            wu_sb = consts.tile([P, QCH], f16)
            tiny = consts.tile([P, 1], f32)
            tiny2 = consts.tile([P, 1], f32)

            # first weights on the fast queue (gates the first matmul),
            # remaining constants on the SWDGE queue behind it
            nc.sync.dma_start(wq_sb, wq_d)
            nc.gpsimd.dma_start(wk_sb, wk_d)
            nc.gpsimd.dma_start(mask_sb, maskT_d)
            nc.gpsimd.dma_start(wv_sb, wv_d)
            nc.gpsimd.dma_start(wo_sb, wo_d)
            nc.gpsimd.dma_start(bq_sb, bq_d)
            nc.gpsimd.dma_start(bk_sb, bk_d)
            nc.gpsimd.dma_start(bv_sb, bv_d)
            nc.vector.memset(wu_sb, 0.0)
            nc.vector.memset(tiny, 0.0)
            make_identity(nc, ident)
            nc.vector.memset(ones_h, 1.0)
            nc.vector.memset(ones_bc, 1.0)

            # ---- big persistent activations ----
            QT = big.tile([P, M], f16)       # head0 dims on parts 0:64, head1 on 64:128
            KT = big.tile([P, M], f16)
            # token-major V tiles: [tok, mt, 2*(64 cols + ones col)]
            Vtm = big.tile([P, M // P, 2 * (D + 1)], f16)
            YT = big.tile([P, M], f16)       # stacked attention output^T (both heads)

            ones_col = ones_h[:, 0 : M // P].rearrange("p (a b) -> p a b", b=1)
            nc.vector.tensor_copy(Vtm[:, :, D : D + 1], ones_col)
            nc.vector.tensor_copy(Vtm[:, :, 2 * D + 1 : 2 * D + 2], ones_col)

            Exp = mybir.ActivationFunctionType.Exp

            # PE warmup: junk matmuls on a memset tile so the HAM clock
            # reaches full rate before the first real matmul (cold-start
            # matmuls otherwise run ~2-4x slower); also preload the ACT
            # exp table (one-time ~1.5us) while DMA queues spin up.
            wu_ps = psum_f.tile([P, QCH], f32, tag="f", name="wu")
            for _ in range(12):
                nc.tensor.matmul(wu_ps, wu_sb[:, 0:P], wu_sb, start=True, stop=True)
            nc.scalar.activation(tiny2, tiny, Exp, bias=0.0, scale=1.0)

            # ---- projection pieces: each token chunk mc yields small
            # filler closures (~2 matmuls each) so pops never delay the
            # exp stream by more than ~0.5us ----
            xts_by_mc = {}

            def make_pieces(mc, dual_queue=False):
                """Returns (kx_pieces[5] (prefetch + 4), vt_pieces[8],
                q_pieces[4])."""
                msl = bass.ts(mc, MCH)
                st = {}

                def pf(mc=mc, msl=msl):
                    # DMA-issue only: popped several tiles ahead of the
                    # matmuls so the x chunk is resident when they run
                    xts = []
                    for kc in range(KC):
                        xt = xt_pool.tile([P, MCH], f16, tag="xt", name="xt")
                        eng = nc.scalar if (dual_queue and kc % 2) else nc.sync
                        eng.dma_start(xt, xT_d[bass.ts(kc, P), msl])
                        xts.append(xt)
                    xts_by_mc[mc] = xts

                def kx(i, mc=mc, msl=msl, st=st):
                    if i == 0:
                        st["kp"] = psum_f.tile([P, MCH], f32, tag="f", name="kp")
                    for kc in (2 * i, 2 * i + 1):
                        nc.tensor.matmul(
                            st["kp"], wk_sb[:, kc, :], xts_by_mc[mc][kc],
                            start=(kc == 0), stop=(kc == KC - 1),
                        )
                    if i == 3:
                        nc.vector.tensor_scalar_add(KT[:, msl], st["kp"], bk_sb)

                def vt(i, mc=mc, msl=msl, st=st):
                    if i < 4:
                        if i == 0:
                            st["vp"] = psum_f.tile([P, MCH], f32, tag="f", name="vp")
                        for kc in (2 * i, 2 * i + 1):
                            nc.tensor.matmul(
                                st["vp"], wv_sb[:, kc, :], xts_by_mc[mc][kc],
                                start=(kc == 0), stop=(kc == KC - 1),
                            )
                        if i == 3:
                            v = vt_pool.tile([P, MCH], f16, name="vt", tag="vt")
                            nc.vector.tensor_scalar_add(v, st["vp"], bv_sb)
                            st["vt"] = v
                    else:
                        j = i - 4
                        mt = mc * (MCH // P) + j
                        vtp = psum_f.tile([P, P], f16, tag="f", name="vtp")
                        nc.tensor.transpose(vtp, st["vt"][:, bass.ts(j, P)], ident)
                        nc.vector.tensor_copy(Vtm[:, mt, 0:D], vtp[:, 0:D])
                        nc.vector.tensor_copy(
                            Vtm[:, mt, D + 1 : 2 * D + 1], vtp[:, D : 2 * D]
                        )

                def q(i, mc=mc, msl=msl, st=st):
                    if i == 0:
                        st["qp"] = psum_f.tile([P, MCH], f32, tag="f", name="qp")
                    for kc in (2 * i, 2 * i + 1):
                        nc.tensor.matmul(
                            st["qp"], wq_sb[:, kc, :], xts_by_mc[mc][kc],
                            start=(kc == 0), stop=(kc == KC - 1),
                        )
                    if i == 3:
                        nc.vector.tensor_scalar_add(QT[:, msl], st["qp"], bq_sb)
                        del xts_by_mc[mc]

                KXp = [pf] + [(lambda i=i, f=kx: f(i)) for i in range(4)]
                VTp = [(lambda i=i, f=vt: f(i)) for i in range(8)]
                Qp = [(lambda i=i, f=q: f(i)) for i in range(4)]
                return KXp, VTp, Qp

            pieces = [make_pieces(mc, dual_queue=(mc == 0)) for mc in range(NMC)]
            # batch-0 chunk 0: x DMAs first (overlap the warmup), then only
            # what pass-0's first tiles need (K, Q, V matmuls); the V
            # transposes drip in as the first filler pieces
            pieces[0][0][0]()                      # prefetch x chunk 0


            # filler order (emission order defines dataflow):
            #   b0 chunks 1-3 K/V just-in-time for pass 0, then b0 Q
            #   chunks, then all b1 chunks, then out-proj tiles (appended
            #   as their normalization completes)
            for p in pieces[0][0][1:] + pieces[0][1][0:4] + pieces[0][2]:
                p()

            filler = []
            idx = {}

            def add(name, ps):
                idx[name] = len(filler) + len(ps)   # index AFTER this group
                filler.extend(ps)

            add("vt0", pieces[0][1][4:])
            for mc in (1, 2, 3):
                add(f"kx{mc}", pieces[mc][0])
                add(f"vt{mc}", pieces[mc][1])
            for mc in (1, 2, 3):
                add(f"q{mc}", pieces[mc][2])
            for mc in (4, 5, 6, 7):
                add(f"kx{mc}", pieces[mc][0])
                add(f"vt{mc}", pieces[mc][1])
                add(f"q{mc}", pieces[mc][2])
            popped = [0]

            def pop(n=1):
                for _ in range(n):
                    if filler:
                        filler.pop(0)()
                        popped[0] += 1

            def pop_until(k):
                while popped[0] < k and filler:
                    pop()

            # just-in-time demand: before emitting iteration (pi, t),
            # this many filler pieces must have been emitted (emission
            # order defines the data a read sees).
            def demand(pi, t):
                b, qc = passes[pi]
                need = 0
                # Q chunks consumed by this pass's scores
                qmax = qc if b == 0 else 4 + qc
                for k in range(1, qmax + 1):
                    need = max(need, idx[f"q{k}"])
                # progressive K/V within the first pass of each batch:
                # scores(t) reads KT chunk ck; the attn@V flushed at this
                # iteration reads Vtm transpose piece for tile t-1 (use t:
                # one ahead, also covers the cross-pass flush)
                if qc == 0:
                    lo = 1 if b == 0 else 4
                    hi = 3 if b == 0 else 7
                    ck = t // 4 + 4 * b
                    if b == 0 and ck == 0:
                        need = max(need, idx["vt0"] - 4 + (t % 4) + 1)
                    if lo <= ck:
                        need = max(need, idx[f"kx{ck}"])
                        base = idx[f"vt{ck}"] - 8
                        need = max(need, base + 4 + (t % 4) + 1)
                    # prefetch: next chunk's x DMAs issued 4 tiles ahead
                    ck2 = ck + 1
                    if lo <= ck2 <= hi:
                        need = max(need, idx[f"kx{ck2}"] - 4)
                # prefetch chunk 4's x before the b0->b1 transition
                if (b, qc) == (0, 3) and t >= 12:
                    need = max(need, idx["kx4"] - 4)
                return need

            def emit_norm_h(b, qc, h, av_sb):
                qsl = bass.ds(b * S + qc * QCH, QCH)
                # partition-broadcast the raw sums [1,512] -> [64,512] with
                # a K=1 PE matmul (ones stationary), then reciprocal at base
                # partition 0 (custom-DVE approx ops misbehave at base 64)
                # and the normalize multiply on the otherwise-idle GPSIMD.
                s16 = r_pool.tile([1, QCH], f16, tag="s16", bufs=4, name="s16")
                nc.vector.tensor_copy(s16, av_sb[D : D + 1, :])
                bc = psum_f.tile([D, QCH], f32, tag="f", name="bc")
                nc.tensor.matmul(bc, ones_bc[0:1, :], s16, start=True, stop=True)
                rbs = r_pool.tile([D, QCH], f32, tag="rbs", bufs=4, name="rbs")
                rsc2 = r_pool.tile([D, QCH], f32, tag="rsc2", name="rsc2")
                nc.vector.reciprocal_approx_accurate(rbs, bc, rsc2)
                nc.vector.tensor_mul(
                    YT[bass.ds(D * h, D), qsl], av_sb[0:D, :], rbs
                )

            def emit_norm(b, qc, av_sbs):
                for h in range(2):
                    emit_norm_h(b, qc, h, av_sbs[h])

            def emit_outproj_ec(b, j, ec, spare_psum=False, stage_eng=None):
                m0 = b * S + j * P
                esl = bass.ts(ec, 512)
                if spare_psum:
                    op = psum_sc.tile([P, 512], f32, tag="sc", name="op_s")
                else:
                    op = psum_f.tile([P, 512], f32, tag="f", name="op")
                nc.tensor.matmul(
                    op, YT[:, bass.ds(m0, P)], wo_sb[:, esl], start=True, stop=True
                )
                osb = out_pool.tile([P, 512], f16, name="osb")
                if stage_eng == "scalar":
                    nc.scalar.copy(osb, op)
                else:
                    nc.vector.tensor_copy(osb, op)
                nc.sync.dma_start(out_d[bass.ds(m0, P), esl], osb)

            def outproj_pieces(b, jlo, jhi):
                return [
                    (lambda bb=b, jj=j, ee=ec: emit_outproj_ec(bb, jj, ee))
                    for j in range(jlo, jhi)
                    for ec in range(E // 512)
                ]

            # ---- attention passes, attn@V pipelined one tile behind ----
            passes = [(b, qc) for b in range(B) for qc in range(S // QCH)]
            pending = []
            done = {0: set(), 1: set()}
            deferred = [None]

            def flush_av():
                if deferred[0] is not None:
                    deferred[0]()
                    deferred[0] = None

            for pi, (b, qc) in enumerate(passes):
                qsl = bass.ds(b * S + qc * QCH, QCH)
                avs = [
                    psum_av.tile([D + 1, QCH], f32, tag="av", name="av")
                    for _ in range(2)
                ]
                for t in range(NKT):
                    pop_until(demand(pi, t))
                    ksl = bass.ds(b * S + t * P, P)
                    bt = b * 16 + t
                    # scores for both heads concurrently: PE row tiles
                    # (0,0) and (64,0) writing adjacent PSUM banks
                    sc = psum_sc.tile([P, 2 * QCH], f32, tag="sc", name="sc")
                    nc.tensor.matmul(
                        sc[:, 0:QCH], KT[0:D, ksl], QT[0:D, qsl],
                        start=True, stop=True,
                    )
                    nc.tensor.matmul(
                        sc[:, QCH : 2 * QCH], KT[D:P, ksl], QT[D:P, qsl],
                        start=True, stop=True,
                    )
                    # one exp() covers both heads: the key-padding bias
                    # is per-key (partition), identical for both heads
                    pt = pt_pool.tile([P, 2 * QCH], f16, tag="pt", name="pt")
                    nc.scalar.activation(
                        pt, sc, Exp, bias=mask_sb[:, bt : bt + 1], scale=1.0
                    )
                    # attn@V of the PREVIOUS tile (its exp is long done, so
                    # the in-order PE stream never waits on ACT here)
                    flush_av()

                    def av_emit(avs=avs, bt=bt, pt=pt, first=(t == 0),
                                last=(t == NKT - 1), pp=(b, qc)):
                        nc.tensor.matmul(
                            avs[0], Vtm[:, bt, 0 : D + 1], pt[:, 0:QCH],
                            start=first, stop=last,
                        )
                        nc.tensor.matmul(
                            avs[1], Vtm[:, bt, D + 1 : 2 * (D + 1)],
                            pt[:, QCH : 2 * QCH],
                            start=first, stop=last,
                        )
                        if last:
                            av_sbs = []
                            for h in range(2):
                                av_sb = r_pool.tile(
                                    [D + 1, QCH], f32, tag="avsb", bufs=4,
                                    name="avsb",
                                )
                                nc.vector.tensor_copy(av_sb, avs[h])
                                av_sbs.append(av_sb)
                            pending.append((pp[0], pp[1], av_sbs))

                    deferred[0] = av_emit
                    # previous pass's normalization (DVE/DMA only)
                    if t == 1 and pending:
                        nb, nqc, nav = pending.pop(0)
                        emit_norm(nb, nqc, nav)
                        done[nb].add(nqc)
                        if nb == 0 and len(done[0]) == 4:
                            filler.extend(outproj_pieces(0, 0, S // P))
                        elif nb == 1 and nqc < 2:
                            filler.extend(outproj_pieces(1, 4 * nqc, 4 * (nqc + 1)))
                        # qc2's tiles are reserved to hide the tail norm
                    # steady drip of filler into the per-tile PE slack;
                    # passes 1-3 drain harder so the b1 projections are
                    # fully resident before the b0->b1 transition
                    if t not in (0, NKT - 1):
                        pop(2 if 1 <= pi <= 3 else 1)
            # tail: flush the last attn@V, issue the last normalization's
            # DMA/reciprocal chain, hide its latency behind the remaining
            # filler, then the final out-proj tiles it unblocks
            flush_av()
            b_l, qc_l, av_sbs_l = pending.pop(0)   # (1, 3)
            emit_norm_h(b_l, qc_l, 0, av_sbs_l[0])
            emit_norm_h(b_l, qc_l, 1, av_sbs_l[1])
            while filler:
                pop()
            k = 0
            for j in range(8, S // P):
                for ec in range(E // 512):
                    emit_outproj_ec(
                        1, j, ec,
                        spare_psum=(j >= 12 and ec % 2 == 1),
                        stage_eng="scalar" if k % 2 else None,
                    )
                    k += 1

    nc.compile()
    return nc


def kernel(x, mask, Wq, bq, Wk, bk, Wv, bv, Wo, bo):
    global LAST_RESULTS, _PROGRAM
    from concourse.bass_utils import run_bass_kernel_spmd

    if _PROGRAM is None:
        _PROGRAM = _build_program()
    nc = _PROGRAM

    f16 = np.float16
    x = np.asarray(x, dtype=np.float32)
    mask = np.asarray(mask)
    f32c = lambda a: np.ascontiguousarray(np.asarray(a, dtype=np.float32))

    xT = np.ascontiguousarray(x.reshape(M, E).T.astype(f16))     # [E, M]
    maskf = np.where(mask, np.float32(NEG), np.float32(0.0)).astype(np.float32)
    maskT = np.ascontiguousarray(
        maskf.reshape(B, 16, P).transpose(2, 0, 1).reshape(P, B * 16)
    )
    scale = np.float32(1.0 / np.sqrt(D))

    in_maps = []
    for c in range(NCORES):
        csl = slice(P * c, P * (c + 1))
        wq_c = (np.asarray(Wq, dtype=np.float32)[:, csl] * scale).astype(f16)
        wk_c = np.asarray(Wk, dtype=np.float32)[:, csl].astype(f16)
        wv_c = np.asarray(Wv, dtype=np.float32)[:, csl].astype(f16)
        in_maps.append(
            {
                "xT": xT,
                "wq": np.ascontiguousarray(wq_c.reshape(KC, P, P).transpose(1, 0, 2)),
                "wk": np.ascontiguousarray(wk_c.reshape(KC, P, P).transpose(1, 0, 2)),
                "wv": np.ascontiguousarray(wv_c.reshape(KC, P, P).transpose(1, 0, 2)),
                "wo": np.ascontiguousarray(
                    np.asarray(Wo, dtype=np.float32)[csl, :].astype(f16)
                ),
                "bq": f32c(np.asarray(bq)[csl] * scale).reshape(P, 1),
                "bk": f32c(np.asarray(bk)[csl]).reshape(P, 1),
                "bv": f32c(np.asarray(bv)[csl]).reshape(P, 1),
                "maskT": maskT,
            }
        )

    trace = bool(os.environ.get("KERNEL_TRACE"))
    LAST_RESULTS = run_bass_kernel_spmd(
        nc, in_maps, list(range(NCORES)), trace=trace
    )

    acc = np.zeros((M, E), dtype=np.float32)
    for res in LAST_RESULTS.results:
        acc += res["out"].astype(np.float32)
    out = (acc + np.asarray(bo, dtype=np.float32)[None, :]).astype(np.float32)
    return out.reshape(B, S, E)
